# revision 13
# baseline (speedup 1.0000x reference)
"""Trainium2 Bass kernel for the Wilson-Cowan rate recurrence.

    Phi(x) = M*x/(x^2+sigma^2) * relu(x+th)
    nu_{t+1} = nu_t + dt/tau * (-nu_t + Phi(E_t - r*nu_t))
    E: [8, 4096, 1024] f32; params [1024]; out nu trajectory [8, 4096, 1024].

Strategy
--------
The scan is elementwise over (batch, unit): 8192 independent length-4096
nonlinear recurrences. The backend cost is dominated by a fixed per-
instruction overhead, so the kernel minimizes instruction count:

* Sharding: core c owns units [128c, 128c+128) (partition = unit, so all
  per-unit params are [P,1] per-partition scalars), all batches, all time.
* Time-parallel chunking: the state forgets its past exponentially
  (Jacobian 1 - c(1 + r*Phi'), c = dt/tau in [0.05, 0.1]); the time axis
  is cut into KC chunks run in lockstep side-by-side in the free dim,
  each warmed up from z=0 for W extra steps (measured end-to-end rel err:
  W=128 -> 1.6e-4, W=160 -> ~2e-5, W=192 -> ~3e-6).
* State transform z = r*nu (host divides by r; min r ~1.5e-3) gives a
  2-scalar-slot-friendly update with per-partition constants
  a = 1-c, rcM = r*c*M, th, sig2:
      u = e - z; w = relu(u+th); z' = a*z + rcM*u*w / (u^2+sig2)
* 8 DVE instructions per step, single engine (no cross-engine sync):
  TT/STT/reciprocal only. E is uploaded once (t-major, zero-padded by W)
  and each SBUF block is gathered with a strided 3-dim DMA, so chunk
  warmup overlap costs device DMA reads, not host uploads.
"""
import sys
sys.path.insert(0, "/opt/trn_rl_repo")
import numpy as np

import concourse.bass as bass
import concourse.mybir as mybir
from concourse.bass_utils import run_bass_kernel_spmd

DT = np.float32(0.1)
B, T, N = 8, 4096, 1024
P = 128                    # partitions = units per core
NCORES = 8

# tunables
KC = 128                   # time chunks per core
W = 160                    # warmup steps per chunk
SB = 8                     # steps per DMA block

L = T // KC                # chunk length
S = L + W                  # compute steps per core
FD = 8 * KC                # free dim per step tile (chunk-major: k, b)
NB = S // SB               # DMA blocks
WB = W // SB               # warmup blocks (not DMA'd out)
TE = W + T                 # padded E length (in steps)
assert L * KC == T and W % SB == 0 and S % SB == 0

f32 = mybir.dt.float32


def build_kernel(repeat=1, timing=False):
    """timing=True shrinks DRAM I/O (results bogus, compute identical) so
    repeat-subtraction wall-clock isolates device time."""
    ALU = mybir.AluOpType
    nc = bass.Bass()
    if timing:
        e_in = nc.declare_dram_parameter("e", [P, SB * FD], f32, isOutput=False)
        z_out = nc.declare_dram_parameter("zout", [P, SB * FD], f32, isOutput=True)

        def e_src(b):
            return e_in[:, :]

        def o_dst(b):
            return z_out[:, :]
    else:
        e_in = nc.declare_dram_parameter("e", [P, TE * 8], f32, isOutput=False)
        z_out = nc.declare_dram_parameter("zout", [P, L * FD], f32, isOutput=True)

        def e_src(b):
            # block gather [p][k: stride L*8][(s b): SB*8-contig], t' = k*L + b*SB + s
            src = e_in[:, :].copy()
            src.ap = mybir.VecI64Pair([(TE * 8, P), (L * 8, KC), (1, SB * 8)])
            src.offset = (b % NB) * SB * 8
            return src

        def o_dst(b):
            bo = (b % NB) - WB
            return z_out[:, bo * SB * FD:(bo + 1) * SB * FD]

    par_in = nc.declare_dram_parameter("par", [P, 4], f32, isOutput=False)

    with (
        nc.sbuf_tensor([P, SB * FD], f32) as stg,
        nc.sbuf_tensor([P, SB * FD], f32) as eb0,
        nc.sbuf_tensor([P, SB * FD], f32) as eb1,
        nc.sbuf_tensor([P, SB * FD], f32) as ob0,
        nc.sbuf_tensor([P, SB * FD], f32) as ob1,
        nc.sbuf_tensor([P, 4], f32) as pt,
        nc.sbuf_tensor([P, FD], f32) as zprev,
        nc.sbuf_tensor([P, FD], f32) as zt,  # zeros
        nc.sbuf_tensor([P, FD], f32) as ut,
        nc.sbuf_tensor([P, FD], f32) as wt,
        nc.sbuf_tensor([P, FD], f32) as pdt,
        nc.sbuf_tensor([P, FD], f32) as dsq,
        nc.sbuf_tensor([P, FD], f32) as d2t,
        nc.sbuf_tensor([P, FD], f32) as rect,
        nc.sbuf_tensor([P, FD], f32) as Zt,
        nc.semaphore() as se,   # staging loads (+16 each)
        nc.semaphore() as sy,   # DVE staging->eblk copy completions (+1)
        nc.semaphore() as sc,   # DVE block completions (+1)
        nc.semaphore() as sz,   # out-DMA completions (+16)
        nc.Block() as block,
    ):
        ebufs = [eb0, eb1]
        obufs = [ob0, ob1]
        NG = repeat * NB

        @block.sync
        def _(sync):
            nout = 0
            sync.dma_start(out=pt[:], in_=par_in[:]).then_inc(se, 16)
            sync.dma_start(out=stg[:], in_=e_src(0)).then_inc(se, 16)
            for r_ in range(repeat):
                for b in range(NB):
                    g = r_ * NB + b
                    # reload staging for block g+1 once copy(g) freed it
                    if g + 1 < NG:
                        sync.wait_ge(sy, g + 1)
                        sync.dma_start(out=stg[:], in_=e_src(b + 1)).then_inc(se, 16)
                    sync.wait_ge(sc, g + 1)
                    if b >= WB:
                        sync.dma_start(out=o_dst(b), in_=obufs[b % 2][:]).then_inc(sz, 16)
                        nout += 1
            sync.wait_ge(sz, 16 * nout)

        @block.vector
        def _(vector):
            th, rcM, sig2, a = (pt[:, i:i + 1] for i in range(4))
            vector.wait_ge(se, 16)
            nc.vector.memset(zprev[:], 0.0)
            nc.vector.memset(zt[:], 0.0)
            # staging [p][k][s][b] viewed step-major; eblk stored step-major
            stg_p = stg[:].rearrange("p (k s b) -> p s k b", k=KC, s=SB)

            # out-DMA ordinal per global block (absent = warmup, no DMA)
            out_ord = {}
            _o = 0
            for g in range(NG):
                if g % NB >= WB:
                    _o += 1
                    out_ord[g] = _o
            for r_ in range(repeat):
                zp = zprev[:]
                for b in range(NB):
                    g = r_ * NB + b
                    # rearrange-copy staging -> eblk for this block
                    vector.wait_ge(se, 16 * (g + 2))
                    et = ebufs[b % 2]
                    nc.vector.tensor_copy(
                        et[:].rearrange("p (s k b) -> p s k b", s=SB, k=KC),
                        stg_p).then_inc(sy, 1)
                    if g >= 2 and out_ord.get(g - 2):
                        vector.wait_ge(sz, 16 * out_ord[g - 2])
                    ot = obufs[b % 2]
                    for s_ in range(SB):
                        es = et[:, s_ * FD:(s_ + 1) * FD]
                        zs = ot[:, s_ * FD:(s_ + 1) * FD]
                        nc.vector.tensor_tensor(
                            out=ut[:], in0=es, in1=zp, op=ALU.subtract)
                        nc.vector.scalar_tensor_tensor(
                            out=wt[:], in0=ut[:], scalar=th, in1=zt[:],
                            op0=ALU.add, op1=ALU.max)
                        nc.vector.tensor_tensor(
                            out=pdt[:], in0=ut[:], in1=wt[:], op=ALU.mult)
                        nc.vector.tensor_tensor(
                            out=dsq[:], in0=ut[:], in1=ut[:], op=ALU.mult)
                        nc.vector.scalar_tensor_tensor(
                            out=d2t[:], in0=dsq[:], scalar=sig2, in1=dsq[:],
                            op0=ALU.add, op1=ALU.max)
                        nc.vector.reciprocal(out=rect[:], in_=d2t[:])
                        nc.vector.scalar_tensor_tensor(
                            out=Zt[:], in0=pdt[:], scalar=rcM, in1=rect[:],
                            op0=ALU.mult, op1=ALU.mult)
                        inst = nc.vector.scalar_tensor_tensor(
                            out=zs, in0=zp, scalar=a, in1=Zt[:],
                            op0=ALU.mult, op1=ALU.add)
                        zp = zs
                    inst.then_inc(sc, 1)

    return nc


_NC_CACHE = {}


def _get_nc(repeat=1, timing=False):
    key = (repeat, timing)
    if key not in _NC_CACHE:
        _NC_CACHE[key] = build_kernel(repeat, timing)
    return _NC_CACHE[key]


def _prep_inputs(E, r, tau_nu, M, sigma, th):
    """Host-side shard + relayout. Returns per-core input maps."""
    E = np.asarray(E, dtype=np.float32)
    r = np.asarray(r, dtype=np.float32)
    tau_nu = np.asarray(tau_nu, dtype=np.float32)
    M = np.asarray(M, dtype=np.float32)
    sigma = np.asarray(sigma, dtype=np.float32)
    th = np.asarray(th, dtype=np.float32)

    c = DT / tau_nu
    a = (1.0 - c).astype(np.float32)
    rcM = (r * c * M).astype(np.float32)
    sig2 = (sigma * sigma).astype(np.float32)

    in_maps = []
    for cidx in range(NCORES):
        nsl = slice(128 * cidx, 128 * (cidx + 1))
        # [P, TE, 8]: W zero-padded steps, then E[b, t, n] -> [n, t, b]
        Ec = np.zeros((P, TE, 8), np.float32)
        Ec[:, W:, :] = E[:, :, nsl].transpose(2, 1, 0)
        par = np.stack([th[nsl], rcM[nsl], sig2[nsl], a[nsl]], axis=1)
        in_maps.append({"e": Ec.reshape(P, TE * 8),
                        "par": np.ascontiguousarray(par)})
    return in_maps


def _post_outputs(results, r):
    """Gather per-core z trajectories into nu [B, T, N]."""
    r = np.asarray(r, dtype=np.float32)
    NBO = NB - WB
    nu = np.empty((B, T, N), dtype=np.float32)
    for cidx in range(NCORES):
        nsl = slice(128 * cidx, 128 * (cidx + 1))
        zc = results[cidx]["zout"].reshape(P, NBO, SB, KC, 8)
        # t = k*L + bo*SB + s  ->  [b, k, bo, s, p]
        zc = zc.transpose(4, 3, 1, 2, 0).reshape(8, T, P)
        nu[:, :, nsl] = zc / r[nsl]
    return nu


def kernel(E, r, tau_nu, M, sigma, th):
    in_maps = _prep_inputs(E, r, tau_nu, M, sigma, th)
    nc = _get_nc(repeat=1)
    res = run_bass_kernel_spmd(nc, in_maps, list(range(NCORES)))
    return _post_outputs(res.results, r)


# revision 14
# speedup vs baseline: 1.4260x; 1.4260x over previous
"""Trainium2 Bass kernel for the Wilson-Cowan rate recurrence.

    Phi(x) = M*x/(x^2+sigma^2) * relu(x+th)
    nu_{t+1} = nu_t + dt/tau * (-nu_t + Phi(E_t - r*nu_t))
    E: [8, 4096, 1024] f32; params [1024]; out nu trajectory [8, 4096, 1024].

Strategy
--------
The scan is elementwise over (batch, unit): 8192 independent length-4096
nonlinear recurrences. The backend cost is dominated by a fixed per-
instruction overhead, so the kernel minimizes instruction count:

* Sharding: core c owns units [128c, 128c+128) (partition = unit, so all
  per-unit params are [P,1] per-partition scalars), all batches, all time.
* Time-parallel chunking: the state forgets its past exponentially
  (Jacobian 1 - c(1 + r*Phi'), c = dt/tau in [0.05, 0.1]); the time axis
  is cut into KC chunks run in lockstep side-by-side in the free dim,
  each warmed up from z=0 for W extra steps (measured end-to-end rel err:
  W=128 -> 1.6e-4, W=160 -> ~2e-5, W=192 -> ~3e-6).
* State transform z = r*nu (host divides by r; min r ~1.5e-3) gives a
  2-scalar-slot-friendly update with per-partition constants
  a = 1-c, rcM = r*c*M, th, sig2:
      u = e - z; w = relu(u+th); z' = a*z + rcM*u*w / (u^2+sig2)
* 8 DVE instructions per step, single engine (no cross-engine sync):
  TT/STT/reciprocal only. E is uploaded once (t-major, zero-padded by W)
  and each SBUF block is gathered with a strided 3-dim DMA, so chunk
  warmup overlap costs device DMA reads, not host uploads.
"""
import sys
sys.path.insert(0, "/opt/trn_rl_repo")
import numpy as np

import concourse.bass as bass
import concourse.mybir as mybir
from concourse.bass_utils import run_bass_kernel_spmd

DT = np.float32(0.1)
B, T, N = 8, 4096, 1024
P = 128                    # partitions = units per core
NCORES = 8

# tunables
KC = 128                   # time chunks per core
W = 144                    # warmup steps per chunk
SB = 8                     # steps per DMA block

L = T // KC                # chunk length
S = L + W                  # compute steps per core
FD = 8 * KC                # free dim per step tile (chunk-major: k, b)
NB = S // SB               # DMA blocks
WB = W // SB               # warmup blocks (not DMA'd out)
TE = W + T                 # padded E length (in steps)
assert L * KC == T and W % SB == 0 and S % SB == 0

f32 = mybir.dt.float32


def build_kernel(repeat=1, timing=False):
    """timing=True shrinks DRAM I/O (results bogus, compute identical) so
    repeat-subtraction wall-clock isolates device time."""
    ALU = mybir.AluOpType
    nc = bass.Bass()
    if timing:
        e_in = nc.declare_dram_parameter("e", [P, SB * FD], f32, isOutput=False)
        z_out = nc.declare_dram_parameter("zout", [P, SB * FD], f32, isOutput=True)

        def e_src(b):
            return e_in[:, :]

        def o_dst(b):
            return z_out[:, :]
    else:
        e_in = nc.declare_dram_parameter("e", [P, TE * 8], f32, isOutput=False)
        z_out = nc.declare_dram_parameter("zout", [P, L * FD], f32, isOutput=True)

        def e_src(b):
            # block gather [p][k: stride L*8][(s b): SB*8-contig], t' = k*L + b*SB + s
            src = e_in[:, :].copy()
            src.ap = mybir.VecI64Pair([(TE * 8, P), (L * 8, KC), (1, SB * 8)])
            src.offset = (b % NB) * SB * 8
            return src

        def o_dst(b):
            bo = (b % NB) - WB
            return z_out[:, bo * SB * FD:(bo + 1) * SB * FD]

    par_in = nc.declare_dram_parameter("par", [P, 4], f32, isOutput=False)

    with (
        nc.sbuf_tensor([P, SB * FD], f32) as stg,
        nc.sbuf_tensor([P, SB * FD], f32) as eb0,
        nc.sbuf_tensor([P, SB * FD], f32) as eb1,
        nc.sbuf_tensor([P, SB * FD], f32) as ob0,
        nc.sbuf_tensor([P, SB * FD], f32) as ob1,
        nc.sbuf_tensor([P, 4], f32) as pt,
        nc.sbuf_tensor([P, FD], f32) as zprev,
        nc.sbuf_tensor([P, FD], f32) as zt,  # zeros
        nc.sbuf_tensor([P, FD], f32) as ut,
        nc.sbuf_tensor([P, FD], f32) as wt,
        nc.sbuf_tensor([P, FD], f32) as pdt,
        nc.sbuf_tensor([P, FD], f32) as dsq,
        nc.sbuf_tensor([P, FD], f32) as d2t,
        nc.sbuf_tensor([P, FD], f32) as rect,
        nc.sbuf_tensor([P, FD], f32) as Zt,
        nc.semaphore() as se,   # staging loads (+16 each)
        nc.semaphore() as sy,   # DVE staging->eblk copy completions (+1)
        nc.semaphore() as sc,   # DVE block completions (+1)
        nc.semaphore() as sz,   # out-DMA completions (+16)
        nc.Block() as block,
    ):
        ebufs = [eb0, eb1]
        obufs = [ob0, ob1]
        NG = repeat * NB

        @block.sync
        def _(sync):
            nout = 0
            sync.dma_start(out=pt[:], in_=par_in[:]).then_inc(se, 16)
            sync.dma_start(out=stg[:], in_=e_src(0)).then_inc(se, 16)
            for r_ in range(repeat):
                for b in range(NB):
                    g = r_ * NB + b
                    # reload staging for block g+1 once copy(g) freed it
                    if g + 1 < NG:
                        sync.wait_ge(sy, g + 1)
                        sync.dma_start(out=stg[:], in_=e_src(b + 1)).then_inc(se, 16)
                    sync.wait_ge(sc, g + 1)
                    if b >= WB:
                        sync.dma_start(out=o_dst(b), in_=obufs[b % 2][:]).then_inc(sz, 16)
                        nout += 1
            sync.wait_ge(sz, 16 * nout)

        @block.vector
        def _(vector):
            th, rcM, sig2, a = (pt[:, i:i + 1] for i in range(4))
            vector.wait_ge(se, 16)
            nc.vector.memset(zprev[:], 0.0)
            nc.vector.memset(zt[:], 0.0)
            # staging [p][k][s][b] viewed step-major; eblk stored step-major
            stg_p = stg[:].rearrange("p (k s b) -> p s k b", k=KC, s=SB)

            # out-DMA ordinal per global block (absent = warmup, no DMA)
            out_ord = {}
            _o = 0
            for g in range(NG):
                if g % NB >= WB:
                    _o += 1
                    out_ord[g] = _o
            for r_ in range(repeat):
                zp = zprev[:]
                for b in range(NB):
                    g = r_ * NB + b
                    # rearrange-copy staging -> eblk for this block
                    vector.wait_ge(se, 16 * (g + 2))
                    et = ebufs[b % 2]
                    nc.vector.tensor_copy(
                        et[:].rearrange("p (s k b) -> p s k b", s=SB, k=KC),
                        stg_p).then_inc(sy, 1)
                    if g >= 2 and out_ord.get(g - 2):
                        vector.wait_ge(sz, 16 * out_ord[g - 2])
                    ot = obufs[b % 2]
                    for s_ in range(SB):
                        es = et[:, s_ * FD:(s_ + 1) * FD]
                        zs = ot[:, s_ * FD:(s_ + 1) * FD]
                        nc.vector.tensor_tensor(
                            out=ut[:], in0=es, in1=zp, op=ALU.subtract)
                        nc.vector.scalar_tensor_tensor(
                            out=wt[:], in0=ut[:], scalar=th, in1=zt[:],
                            op0=ALU.add, op1=ALU.max)
                        nc.vector.tensor_tensor(
                            out=pdt[:], in0=ut[:], in1=wt[:], op=ALU.mult)
                        nc.vector.tensor_tensor(
                            out=dsq[:], in0=ut[:], in1=ut[:], op=ALU.mult)
                        nc.vector.scalar_tensor_tensor(
                            out=d2t[:], in0=dsq[:], scalar=sig2, in1=dsq[:],
                            op0=ALU.add, op1=ALU.max)
                        nc.vector.reciprocal(out=rect[:], in_=d2t[:])
                        nc.vector.scalar_tensor_tensor(
                            out=Zt[:], in0=pdt[:], scalar=rcM, in1=rect[:],
                            op0=ALU.mult, op1=ALU.mult)
                        inst = nc.vector.scalar_tensor_tensor(
                            out=zs, in0=zp, scalar=a, in1=Zt[:],
                            op0=ALU.mult, op1=ALU.add)
                        zp = zs
                    inst.then_inc(sc, 1)

    return nc


_NC_CACHE = {}


def _get_nc(repeat=1, timing=False):
    key = (repeat, timing)
    if key not in _NC_CACHE:
        _NC_CACHE[key] = build_kernel(repeat, timing)
    return _NC_CACHE[key]


def _prep_inputs(E, r, tau_nu, M, sigma, th):
    """Host-side shard + relayout. Returns per-core input maps."""
    E = np.asarray(E, dtype=np.float32)
    r = np.asarray(r, dtype=np.float32)
    tau_nu = np.asarray(tau_nu, dtype=np.float32)
    M = np.asarray(M, dtype=np.float32)
    sigma = np.asarray(sigma, dtype=np.float32)
    th = np.asarray(th, dtype=np.float32)

    c = DT / tau_nu
    a = (1.0 - c).astype(np.float32)
    rcM = (r * c * M).astype(np.float32)
    sig2 = (sigma * sigma).astype(np.float32)

    in_maps = []
    for cidx in range(NCORES):
        nsl = slice(128 * cidx, 128 * (cidx + 1))
        # [P, TE, 8]: W zero-padded steps, then E[b, t, n] -> [n, t, b]
        Ec = np.zeros((P, TE, 8), np.float32)
        Ec[:, W:, :] = E[:, :, nsl].transpose(2, 1, 0)
        par = np.stack([th[nsl], rcM[nsl], sig2[nsl], a[nsl]], axis=1)
        in_maps.append({"e": Ec.reshape(P, TE * 8),
                        "par": np.ascontiguousarray(par)})
    return in_maps


def _post_outputs(results, r):
    """Gather per-core z trajectories into nu [B, T, N]."""
    r = np.asarray(r, dtype=np.float32)
    NBO = NB - WB
    nu = np.empty((B, T, N), dtype=np.float32)
    for cidx in range(NCORES):
        nsl = slice(128 * cidx, 128 * (cidx + 1))
        zc = results[cidx]["zout"].reshape(P, NBO, SB, KC, 8)
        # t = k*L + bo*SB + s  ->  [b, k, bo, s, p]
        zc = zc.transpose(4, 3, 1, 2, 0).reshape(8, T, P)
        nu[:, :, nsl] = zc / r[nsl]
    return nu


def kernel(E, r, tau_nu, M, sigma, th):
    in_maps = _prep_inputs(E, r, tau_nu, M, sigma, th)
    nc = _get_nc(repeat=1)
    res = run_bass_kernel_spmd(nc, in_maps, list(range(NCORES)))
    return _post_outputs(res.results, r)
